# revision 1
# baseline (speedup 1.0000x reference)
"""Trainium2 Bass kernel for nn_Attention_31215822307478.

EfficientViT-style attention block:
  qkv 1x1 conv + BN -> split q,k,v -> depthwise 3x3 + BN on q ->
  8-head attention with positional bias over N=784 tokens ->
  ReLU -> 1x1 proj + BN.

Strategy (per core, data-parallel over batch, 4 images/core):
  - BN folded into conv weights/biases on host.
  - q/k produced in head-padded layout (each 16-ch head at a 32-aligned
    partition offset) so QK^T matmuls can slice operands directly.
  - V produced pre-transposed ([spatial, ch]) straight from the 1x1 conv by
    swapping matmul operand roles; a fused ones-column per head yields the
    softmax denominator for free during the AV matmul.
  - Softmax is max-free (logits are provably small); positional bias enters
    as a host-precomputed exp(bias) table (bf16) via one elementwise multiply.
  - Depthwise 3x3 runs on the PE as 9 accumulated diagonal matmuls over a
    zero-padded [128, 30x30] buffer.
  - Division by the denominator: Z rows stashed at 32-strided partitions of
    one tile (via the PSUM-input scalar_tensor_tensor partition-shift, the
    only DVE pattern that crosses partitions reliably on HW), two batched
    DVE reciprocals per batch, then a K=1 ones-matmul broadcasts 1/Z across
    partitions in PSUM for an in-place multiply. All matmul operands bf16
    (PSUM accumulation stays fp32).
"""

import os
import sys

import numpy as np

for _p in ("/opt/trn_rl_repo", "/root/.axon_site/_ro/trn_rl_repo"):
    if os.path.isdir(_p) and _p not in sys.path:
        sys.path.insert(0, _p)

import ml_dtypes  # noqa: E402
from contextlib import ExitStack  # noqa: E402

import concourse.bass as bass  # noqa: E402
import concourse.mybir as mybir  # noqa: E402
import concourse.tile as tile  # noqa: E402
from concourse import bacc  # noqa: E402
from concourse.alu_op_type import AluOpType  # noqa: E402
from concourse.bass_utils import run_bass_kernel_spmd  # noqa: E402

EPS = 1e-5
DIM, KEY_DIM, HEADS = 256, 16, 8
NH_KD, D, DH = 128, 64, 512
B, H, W = 32, 28, 28
N = H * W  # 784
NCORES = 8
BC = B // NCORES  # 4 images per core
SCALE = KEY_DIM ** -0.5

F32 = mybir.dt.float32
BF16 = mybir.dt.bfloat16
AF = mybir.ActivationFunctionType

KT_SIZES = [128] * 6 + [16]  # 784 = 6*128 + 16 spatial/key tiles
CH = [(0, 512), (512, 272)]  # PSUM-bank-aligned free chunks of 784

_PROGRAM_CACHE = {}


def _build_program():
    nc = bacc.Bacc("TRN2", target_bir_lowering=False, debug=False)

    x_d = nc.dram_tensor("x", [BC, 256, N], BF16, kind="ExternalInput").ap()
    wqkT_d = nc.dram_tensor("wqkT", [2, 128, 512], BF16, kind="ExternalInput").ap()
    wvT_d = nc.dram_tensor("wvT", [2, 128, 520], BF16, kind="ExternalInput").ap()
    wpT_d = nc.dram_tensor("wpT", [8, 64, 256], BF16, kind="ExternalInput").ap()
    dtap_d = nc.dram_tensor("dtaps", [2, 128, 9 * 128], BF16, kind="ExternalInput").ap()
    bias_d = nc.dram_tensor("biases", [128, 8], F32, kind="ExternalInput").ap()
    bvb_d = nc.dram_tensor("bv_bcast", [128, 520], BF16, kind="ExternalInput").ap()
    eb_d = nc.dram_tensor("eb", [128, HEADS * 7 * N], BF16, kind="ExternalInput").ap()
    id_d = nc.dram_tensor("ident", [128, 128], BF16, kind="ExternalInput").ap()
    ones_d = nc.dram_tensor("onesm", [128, 64], F32, kind="ExternalInput").ap()
    ones1_d = nc.dram_tensor("ones1", [1, N], F32, kind="ExternalInput").ap()
    out_d = nc.dram_tensor("out", [BC, 256, N], F32, kind="ExternalOutput").ap()

    with tile.TileContext(nc) as tc, ExitStack() as ctx:
        const = ctx.enter_context(tc.tile_pool(name="const", bufs=1))
        pspool = ctx.enter_context(tc.tile_pool(name="ps", bufs=2, space="PSUM"))
        upool = ctx.enter_context(tc.tile_pool(name="ups", bufs=2, space="PSUM"))
        xpool = ctx.enter_context(tc.tile_pool(name="xp", bufs=3))
        qpadp = ctx.enter_context(tc.tile_pool(name="qpadp", bufs=3))
        kpool = ctx.enter_context(tc.tile_pool(name="kp", bufs=5))
        qdpool = ctx.enter_context(tc.tile_pool(name="qdp", bufs=5))
        vpool = ctx.enter_context(tc.tile_pool(name="vp", bufs=14))
        epool = ctx.enter_context(tc.tile_pool(name="ep", bufs=4))
        apool = ctx.enter_context(tc.tile_pool(name="ap", bufs=4))
        rpool = ctx.enter_context(tc.tile_pool(name="rp", bufs=2))
        rtpool = ctx.enter_context(tc.tile_pool(name="rtp", bufs=5))
        opool = ctx.enter_context(tc.tile_pool(name="op", bufs=2))

        # ---- constants ----
        wqkT = []
        wvT = []
        wpT = []
        dtap = []
        for ki in range(2):
            t = const.tile([128, 512], BF16, tag=f"wqkT{ki}", name=f"wqkT{ki}")
            nc.sync.dma_start(t[:], wqkT_d[ki])
            wqkT.append(t)
        for ki in range(2):
            t = const.tile([128, 520], BF16, tag=f"wvT{ki}", name=f"wvT{ki}")
            nc.sync.dma_start(t[:], wvT_d[ki])
            wvT.append(t)
        for ki in range(8):
            t = const.tile([64, 256], BF16, tag=f"wpT{ki}", name=f"wpT{ki}")
            nc.sync.dma_start(t[:], wpT_d[ki])
            wpT.append(t)
        for g in range(2):
            t = const.tile([128, 9 * 128], BF16, tag=f"dtap{g}", name=f"dtap{g}")
            nc.sync.dma_start(t[:], dtap_d[g])
            dtap.append(t)
        biases = const.tile([128, 8], F32, tag="biases", name="biases")
        nc.sync.dma_start(biases[:], bias_d[:])
        bvb = const.tile([128, 520], BF16, tag="bvb", name="bvb")
        nc.sync.dma_start(bvb[:], bvb_d[:])
        ident = const.tile([128, 128], BF16, tag="ident", name="ident")
        nc.sync.dma_start(ident[:], id_d[:])
        onesm = const.tile([128, 64], F32, tag="onesm", name="onesm")
        nc.sync.dma_start(onesm[:], ones_d[:])
        ones1 = const.tile([1, N], F32, tag="ones1", name="ones1")
        nc.sync.dma_start(ones1[:], ones1_d[:])
        eb = const.tile([128, HEADS * 7 * N], BF16, tag="eb", name="eb")

        st = [dict() for _ in range(BC)]  # per-batch tile state

        def phase_a_chunk(b, c):
            s = st[b]
            if c == 0:
                # x load + qk 1x1 conv into head-padded layout
                s["xb"] = []
                for ki in range(2):
                    t = xpool.tile([128, N], BF16, tag="xb", name="xb")
                    nc.sync.dma_start(t[:], x_d[b, 128 * ki : 128 * (ki + 1), :])
                    s["xb"].append(t)
                s["q_pad"] = []
                s["k_sb"] = []
                for mt in range(4):
                    ps = pspool.tile([128, N], F32, tag="ps", name="ps")
                    for (o, szc) in CH:
                        for ki in range(2):
                            nc.tensor.matmul(
                                ps[:, o : o + szc],
                                wqkT[ki][:, mt * 128 : (mt + 1) * 128],
                                s["xb"][ki][:, o : o + szc],
                                start=(ki == 0),
                                stop=(ki == 1),
                            )
                    if mt < 2:
                        qp = qpadp.tile([128, 900], BF16, tag="qpad", name="qpad")
                        nc.gpsimd.memset(qp[:], 0.0)
                        qp3 = qp[:].rearrange("p (y x) -> p y x", y=30)
                        nc.vector.tensor_scalar_add(
                            qp3[:, 1:29, 1:29],
                            ps[:].rearrange("p (y x) -> p y x", y=28),
                            biases[:, mt : mt + 1],
                        )
                        s["q_pad"].append(qp)
                    else:
                        kt_ = kpool.tile([128, N], BF16, tag="ksb", name="ksb")
                        nc.vector.tensor_scalar_add(
                            kt_[:], ps[:], biases[:, mt : mt + 1]
                        )
                        s["k_sb"].append(kt_)
            elif c == 1:
                # v 1x1 conv, transposed output [spatial, 8*(64+1)]
                s["vt"] = []
                for sp in range(7):
                    ssz = KT_SIZES[sp]
                    psv = pspool.tile([128, 520], F32, tag="ps", name="ps")
                    for (o2, sz2) in [(0, 512), (512, 8)]:
                        for ki in range(2):
                            nc.tensor.matmul(
                                psv[:ssz, o2 : o2 + sz2],
                                s["xb"][ki][:, sp * 128 : sp * 128 + ssz],
                                wvT[ki][:, o2 : o2 + sz2],
                                start=(ki == 0),
                                stop=(ki == 1),
                            )
                    vtt = vpool.tile([128, 520], BF16, tag="vt", name="vt")
                    nc.vector.tensor_tensor(
                        vtt[:ssz, :], psv[:ssz, :], bvb[:ssz, :], op=AluOpType.add
                    )
                    s["vt"].append(vtt)
            else:
                # depthwise 3x3 via 9 diagonal matmuls
                s["q_dw"] = []
                for g in range(2):
                    psd = pspool.tile([128, 1024], F32, tag="ps", name="ps")
                    qp3 = s["q_pad"][g][:].rearrange("p (y x) -> p y x", y=30)
                    for (y0, ny, po) in [(0, 14, 0), (14, 14, 512)]:
                        for t9 in range(9):
                            ty, tx = divmod(t9, 3)
                            nc.tensor.matmul(
                                psd[:, po : po + ny * 28],
                                dtap[g][:, t9 * 128 : (t9 + 1) * 128],
                                qp3[:, y0 + ty : y0 + ty + ny, tx : tx + 28],
                                start=(t9 == 0),
                                stop=(t9 == 8),
                            )
                    qd = qdpool.tile([128, N], BF16, tag="qdw", name="qdw")
                    nc.vector.tensor_scalar_add(
                        qd[:, 0:392], psd[:, 0:392], biases[:, 4 + g : 5 + g]
                    )
                    nc.vector.tensor_scalar_add(
                        qd[:, 392:784], psd[:, 512:904], biases[:, 4 + g : 5 + g]
                    )
                    s["q_dw"].append(qd)

        def phase_b_pair(b, hp):
            s = st[b]
            if hp == 0:
                s["rt"] = []
                s["zs"] = []
                for g2 in range(2):
                    zt = rpool.tile([128, N], F32, tag="zs", name="zs")
                    nc.gpsimd.memset(zt[:], 1.0)
                    s["zs"].append(zt)
            s["rt"].append(rtpool.tile([64, 2 * N], BF16, tag="rt", name="rt"))
            hs = (2 * hp, 2 * hp + 1)
            Us = []
            for h in hs:
                Us.append(upool.tile([65, N], F32, tag="U", name="U"))
            for kt in range(7):
                ksz = KT_SIZES[kt]
                Ss = []
                for hi, h in enumerate(hs):
                    g, j = divmod(h, 4)
                    base = 32 * j
                    S = pspool.tile([128, N], F32, tag="ps", name="ps")
                    for (o, szc) in CH:
                        nc.tensor.matmul(
                            S[:ksz, o : o + szc],
                            s["k_sb"][g][base : base + 16, kt * 128 : kt * 128 + ksz],
                            s["q_dw"][g][base : base + 16, o : o + szc],
                            start=True,
                            stop=True,
                            tile_position=(base, 0),
                        )
                    Ss.append(S)
                for hi, h in enumerate(hs):
                    E = epool.tile([128, N], BF16, tag="E", name="E")
                    nc.scalar.activation(
                        E[:ksz, :], Ss[hi][:ksz, :], AF.Exp, scale=SCALE
                    )
                    A = apool.tile([128, N], BF16, tag="A", name="A")
                    eng = nc.gpsimd if kt in (2, 5) else nc.vector
                    eng.tensor_tensor(
                        A[:ksz, :],
                        E[:ksz, :],
                        eb[:ksz, (h * 7 + kt) * N : (h * 7 + kt + 1) * N],
                        op=AluOpType.mult,
                    )
                    for (o, szc) in CH:
                        nc.tensor.matmul(
                            Us[hi][:, o : o + szc],
                            s["vt"][kt][:ksz, 65 * h : 65 * h + 65],
                            A[:ksz, o : o + szc],
                            start=(kt == 0),
                            stop=(kt == 6),
                        )
            for hi, h in enumerate(hs):
                g2, j2 = divmod(h, 4)
                nc.vector.tensor_scalar_max(
                    s["rt"][hp][:, N * hi : N * hi + N], Us[hi][0:64, :], 0.0
                )
                nc.vector.scalar_tensor_tensor(
                    s["zs"][g2][32 * j2 : 32 * j2 + 1, :],
                    Us[hi][64:65, :],
                    0.0,
                    ones1[:],
                    op0=AluOpType.add,
                    op1=AluOpType.mult,
                )

        def phase_b_div(b):
            s = st[b]
            rz = []
            for g2 in range(2):
                rzt = rpool.tile([128, N], F32, tag="rz", name="rz")
                nc.vector.reciprocal(rzt[:], s["zs"][g2][:])
                rz.append(rzt)
            for hp in range(4):
                for hi in range(2):
                    h = 2 * hp + hi
                    g2, j2 = divmod(h, 4)
                    base = 32 * j2
                    Rb = pspool.tile([64, N], F32, tag="ps", name="ps")
                    for (o, szc) in CH:
                        nc.tensor.matmul(
                            Rb[:, o : o + szc],
                            onesm[base : base + 1, :],
                            rz[g2][base : base + 1, o : o + szc],
                            start=True,
                            stop=True,
                            tile_position=(base, 0),
                        )
                    nc.vector.tensor_tensor(
                        s["rt"][hp][:, N * hi : N * hi + N],
                        s["rt"][hp][:, N * hi : N * hi + N],
                        Rb[:],
                        op=AluOpType.mult,
                    )

        def phase_c(b):
            s = st[b]
            for mt in range(2):
                po_ = pspool.tile([128, N], F32, tag="ps", name="ps")
                for (o, szc) in CH:
                    for ki in range(8):
                        nc.tensor.matmul(
                            po_[:, o : o + szc],
                            wpT[ki][:, mt * 128 : (mt + 1) * 128],
                            s["rt"][ki // 2][:, N * (ki % 2) + o : N * (ki % 2) + o + szc],
                            start=(ki == 0),
                            stop=(ki == 7),
                        )
                ob = opool.tile([128, N], F32, tag="ob", name="ob")
                nc.vector.tensor_scalar_add(ob[:], po_[:], biases[:, 6 + mt : 7 + mt])
                nc.sync.dma_start(out_d[b, mt * 128 : (mt + 1) * 128, :], ob[:])

        # batch 0's convs go first so the PE has work while eb streams in
        # (eb DMAs ride the gpsimd queue so x/weight loads aren't stuck
        # behind 10MB on the sync queue)
        for c in range(3):
            phase_a_chunk(0, c)
        for h in range(HEADS):
            nc.gpsimd.dma_start(
                eb[:, h * 7 * N : (h + 1) * 7 * N],
                eb_d[:, h * 7 * N : (h + 1) * 7 * N],
            )
        for b in range(BC):
            if b + 1 < BC:
                for c in range(3):
                    phase_a_chunk(b + 1, c)
            for hp in range(4):
                phase_b_pair(b, hp)
            phase_b_div(b)
            phase_c(b)

    nc.compile()
    return nc


def get_program():
    if "nc" not in _PROGRAM_CACHE:
        _PROGRAM_CACHE["nc"] = _build_program()
    return _PROGRAM_CACHE["nc"]


def prep_host_inputs(inputs):
    """Fold BN, reorder/pad weights, build exp-bias table. Returns dict of
    np arrays for the non-x DRAM tensors (shared across cores)."""
    f32 = np.float32
    qkv_w = np.asarray(inputs["qkv_w"], f32)[:, :, 0, 0]  # [768, 256]
    s = np.asarray(inputs["qkv_g"], f32) / np.sqrt(np.asarray(inputs["qkv_v"], f32) + EPS)
    Wall = qkv_w * s[:, None]
    ball = np.asarray(inputs["qkv_b"], f32) - np.asarray(inputs["qkv_m"], f32) * s
    Wq, Wk, Wv = Wall[:128], Wall[128:256], Wall[256:]
    bq, bk, bv = ball[:128], ball[128:256], ball[256:]

    qk_pad = np.zeros((512, 256), f32)
    bqk_pad = np.zeros(512, f32)
    for h in range(HEADS):
        g, j = divmod(h, 4)
        qk_pad[128 * g + 32 * j : 128 * g + 32 * j + 16] = Wq[16 * h : 16 * h + 16]
        bqk_pad[128 * g + 32 * j : 128 * g + 32 * j + 16] = bq[16 * h : 16 * h + 16]
        qk_pad[128 * (2 + g) + 32 * j : 128 * (2 + g) + 32 * j + 16] = (
            Wk[16 * h : 16 * h + 16]
        )
        bqk_pad[128 * (2 + g) + 32 * j : 128 * (2 + g) + 32 * j + 16] = (
            bk[16 * h : 16 * h + 16]
        )
    wqkT = np.ascontiguousarray(qk_pad.T).reshape(2, 128, 512)

    vv = np.zeros((520, 256), f32)
    bv_aug = np.zeros(520, f32)
    for h in range(HEADS):
        vv[65 * h : 65 * h + 64] = Wv[64 * h : 64 * h + 64]
        bv_aug[65 * h : 65 * h + 64] = bv[64 * h : 64 * h + 64]
        bv_aug[65 * h + 64] = 1.0
    wvT = np.ascontiguousarray(vv.T).reshape(2, 128, 520)
    bv_bcast = np.ascontiguousarray(np.broadcast_to(bv_aug, (128, 520)))

    s2 = np.asarray(inputs["dw_g"], f32) / np.sqrt(np.asarray(inputs["dw_v"], f32) + EPS)
    dww = np.asarray(inputs["dw_w"], f32)[:, 0] * s2[:, None, None]  # [128,3,3]
    bdw = np.asarray(inputs["dw_b"], f32) - np.asarray(inputs["dw_m"], f32) * s2
    dtaps = np.zeros((2, 128, 9 * 128), f32)
    bdw_pad = np.zeros((2, 128), f32)
    for h in range(HEADS):
        g, j = divmod(h, 4)
        for d in range(16):
            p = 32 * j + d
            c = 16 * h + d
            for t9 in range(9):
                dtaps[g, p, t9 * 128 + p] = dww[c].reshape(9)[t9]
            bdw_pad[g, p] = bdw[c]
    dtaps = dtaps.astype(ml_dtypes.bfloat16)

    sp = np.asarray(inputs["proj_g"], f32) / np.sqrt(
        np.asarray(inputs["proj_v"], f32) + EPS
    )
    Wp = np.asarray(inputs["proj_w"], f32)[:, :, 0, 0] * sp[:, None]  # [256, 512]
    bp = np.asarray(inputs["proj_b"], f32) - np.asarray(inputs["proj_m"], f32) * sp
    wpT = np.ascontiguousarray(Wp.T).reshape(8, 64, 256)

    biases = np.zeros((128, 8), f32)
    biases[:, 0] = bqk_pad[0:128]
    biases[:, 1] = bqk_pad[128:256]
    biases[:, 2] = bqk_pad[256:384]
    biases[:, 3] = bqk_pad[384:512]
    biases[:, 4] = bdw_pad[0]
    biases[:, 5] = bdw_pad[1]
    biases[:, 6] = bp[:128]
    biases[:, 7] = bp[128:]

    ab = np.asarray(inputs["ab"], f32)  # [8, 784]
    idx = np.asarray(inputs["bias_idxs"])  # [784, 784] int32
    ebt = np.exp(ab)[:, idx]  # [8, 784(key), 784(q)] (bias is symmetric)
    eb = np.zeros((128, HEADS * 7 * N), f32)
    for h in range(HEADS):
        for kt in range(7):
            ksz = KT_SIZES[kt]
            blk = ebt[h, kt * 128 : kt * 128 + ksz, :]
            eb[:ksz, (h * 7 + kt) * N : (h * 7 + kt + 1) * N] = blk
    eb = eb.astype(ml_dtypes.bfloat16)

    bf = ml_dtypes.bfloat16
    return {
        "ident": np.eye(128, dtype=np.float32).astype(bf),
        "onesm": np.ones((128, 64), np.float32),
        "ones1": np.ones((1, N), np.float32),
        "wqkT": wqkT.astype(bf),
        "wvT": wvT.astype(bf),
        "wpT": wpT.astype(bf),
        "dtaps": dtaps,
        "biases": biases,
        "bv_bcast": bv_bcast.astype(bf),
        "eb": eb,
    }


def kernel(**inputs):
    nc = get_program()
    shared = prep_host_inputs(inputs)
    x = np.asarray(inputs["x"], np.float32).reshape(B, 256, N).astype(ml_dtypes.bfloat16)
    in_maps = []
    for c in range(NCORES):
        m = dict(shared)
        m["x"] = np.ascontiguousarray(x[BC * c : BC * (c + 1)])
        in_maps.append(m)
    res = run_bass_kernel_spmd(nc, in_maps, core_ids=list(range(NCORES)))
    out = np.concatenate([r["out"] for r in res.results], axis=0)
    return out.reshape(B, 256, H, W)

